# revision 1
# baseline (speedup 1.0000x reference)
"""Converged Toeplitz inhibition kernel for TRN2 (8 NeuronCores, SPMD).

out[n, c, h, w] = sum_k act[n, k, h, w] * Winv[k, c]
where Winv = inv(I - circulant(pad_roll(inhibition_filter, C)))  [C x C]

Strategy (per sharding hint): the tiny C x C inverse is computed on the host
and replicated to every core; activations are sharded along batch N (32 -> 4
per core). Each core runs a dense [K=256] x [M=256] x [N_free=4*4096] matmul:
  - weights held in SBUF as four 128x128 views of one [128, 512] tile
  - activations DMA'd in as [128, 2048] chunks (1 MB) on the SP HWDGE ring
  - PE matmul in float32r (full-rate fp32 path, free dim 512, inputs
    pre-rounded host-side to the fp32r set, ~3.2e-4 scale-relative error)
  - PSUM -> SBUF evacuation alternating ScalarE / VectorE
  - output DMA'd out as [128, 2048] chunks on the ACT HWDGE ring, so the
    read and write streams overlap (~425 GB/s combined vs ~350 single-ring)

Measured on 8 axon trn2 cores: 94-108 us HW exec (phase-dependent HBM-pair
contention), vs ~94 us pure-DMA floor at full pair concurrency.
"""

import numpy as np

import concourse.bass as bass
import concourse.bacc as bacc
import concourse.mybir as mybir
import concourse.tile as tile
from concourse.bass_utils import run_bass_kernel_spmd

N, C, H, W = 32, 256, 64, 64
HW = H * W  # 4096
NCORES = 8
NB = N // NCORES  # batches per core
P = 128  # partitions
FD = 512  # matmul free dim (one fp32 PSUM bank)
NJ = HW // FD  # 8 free-dim chunks per batch

MM_DT = mybir.dt.float32r  # full-rate fp32 matmul path


def _round_fp32r(x: np.ndarray) -> np.ndarray:
    """Round fp32 to the fp32r-representable set (mantissa truncated to 11
    bits, round-to-nearest-even), matching neuron_dtypes' fp32->fp32r cast.

    The PE's fp32r datapath requires operands already rounded; walrus verifies
    the producing instruction rounds, so we round host-side and ship fp32r
    end-to-end.
    """
    u = np.ascontiguousarray(x, dtype=np.float32).view(np.uint32).astype(np.uint64)
    u = (u + ((u >> 12) & 1) + 0x7FF) & 0xFFFFF000
    return u.astype(np.uint32).view(np.float32)


def _build_w(inhibition_filter: np.ndarray) -> np.ndarray:
    """Replicates reference._pad_roll + _circulant + inv(I - tpl) in numpy."""
    filt = np.asarray(inhibition_filter, dtype=np.float32)
    scope = filt.shape[0]
    pad_left = (C - scope) // 2
    padded = np.zeros(C, np.float32)
    padded[pad_left : pad_left + scope] = filt
    kernel = np.roll(padded, C // 2 + 1)
    idx = (np.arange(C)[None, :] - np.arange(C)[:, None]) % C
    tpl = kernel[idx].astype(np.float64)
    w = np.linalg.inv(np.eye(C, dtype=np.float64) - tpl)
    return np.ascontiguousarray(w.astype(np.float32))


def _body(tc: tile.TileContext, out, act, w):
    # In-DMAs ride the SP HWDGE ring (nc.sync), out-DMAs the ACT ring
    # (nc.scalar) so input and output streams don't serialize on one FIFO
    # ring. 1 MB chunks keep the pipeline ends tight.
    nc = tc.nc
    CH = 2048  # chunk width (1 MB tiles), 4 matmul slices per chunk
    NCH = HW // CH  # 2 chunks
    JPC = CH // FD  # 4 matmul free-dim slices per chunk
    with (
        tc.tile_pool(name="wpool", bufs=1) as wpool,
        tc.tile_pool(name="apool", bufs=3) as apool,
        tc.tile_pool(name="opool", bufs=3) as opool,
        tc.tile_pool(name="psum", bufs=8, space="PSUM") as pspool,
    ):
        # One DMA for all four 128x128 weight tiles, on the ACT ring (idle at
        # start) so the SP ring's activation stream isn't stuck behind the
        # fixed completion latency of four tiny transfers.
        wtile = wpool.tile([P, 2 * C], MM_DT, tag="w", name="wtile")
        for k in range(2):
            nc.scalar.dma_start(
                out=wtile[:, k * C : (k + 1) * C], in_=w[k * P : (k + 1) * P, :]
            )
        wt = [
            [wtile[:, k * C + m * P : k * C + (m + 1) * P] for m in range(2)]
            for k in range(2)
        ]

        for n in range(NB):
            a = {}
            for c in range(NCH):
                for k in range(2):
                    a[k, c] = apool.tile([P, CH], MM_DT, tag=f"a{k}{c}", name=f"a{k}{c}")
                    nc.sync.dma_start(
                        out=a[k, c][:],
                        in_=act[n, k * P : (k + 1) * P, c * CH : (c + 1) * CH],
                    )
            for c in range(NCH):
                for m in range(2):
                    o = opool.tile([P, CH], mybir.dt.float32, tag=f"o{m}{c}", name=f"o{m}{c}", bufs=3 if c == 0 else 2)
                    for jj in range(JPC):
                        ps = pspool.tile([P, FD], mybir.dt.float32)
                        nc.tensor.matmul(
                            ps[:],
                            lhsT=wt[0][m],
                            rhs=a[0, c][:, jj * FD : (jj + 1) * FD],
                            start=True,
                            stop=False,
                        )
                        nc.tensor.matmul(
                            ps[:],
                            lhsT=wt[1][m],
                            rhs=a[1, c][:, jj * FD : (jj + 1) * FD],
                            start=False,
                            stop=True,
                        )
                        if jj % 2 == 0:
                            nc.scalar.copy(o[:, jj * FD : (jj + 1) * FD], ps[:])
                        else:
                            nc.vector.tensor_copy(o[:, jj * FD : (jj + 1) * FD], ps[:])
                    nc.scalar.dma_start(
                        out=out[n, m * P : (m + 1) * P, c * CH : (c + 1) * CH],
                        in_=o[:],
                    )


_NC_CACHE = None


def _get_nc():
    global _NC_CACHE
    if _NC_CACHE is None:
        nc = bacc.Bacc(
            "TRN2", debug=False, enable_asserts=False, enable_partition_id=False
        )
        act = nc.dram_tensor("act", [NB, C, HW], MM_DT, kind="ExternalInput").ap()
        w = nc.dram_tensor("w", [C, C], MM_DT, kind="ExternalInput").ap()
        out = nc.dram_tensor("out", [NB, C, HW], mybir.dt.float32, kind="ExternalOutput").ap()
        with tile.TileContext(nc) as tc:
            _body(tc, out, act, w)
        nc.compile()
        _NC_CACHE = nc
    return _NC_CACHE


def _run(activations: np.ndarray, w: np.ndarray, trace: bool = False):
    acts = _round_fp32r(
        np.ascontiguousarray(activations, dtype=np.float32)
    ).reshape(NCORES, NB, C, HW)
    w = _round_fp32r(w)
    in_maps = [{"act": acts[i], "w": w} for i in range(NCORES)]
    nc = _get_nc()
    res = run_bass_kernel_spmd(nc, in_maps, list(range(NCORES)), trace=trace)
    out = np.concatenate([res.results[i]["out"] for i in range(NCORES)], axis=0)
    return out.reshape(N, C, H, W), res


def kernel(activations: np.ndarray, inhibition_filter: np.ndarray) -> np.ndarray:
    w = _build_w(inhibition_filter)
    out, _ = _run(activations, w, trace=False)
    return out



# revision 2
# speedup vs baseline: 1.6756x; 1.6756x over previous
"""Converged Toeplitz inhibition kernel for TRN2 (8 NeuronCores, SPMD).

out[n, c, h, w] = sum_k act[n, k, h, w] * Winv[k, c]
where Winv = inv(I - circulant(pad_roll(inhibition_filter, C)))  [C x C]

Strategy (per sharding hint): the tiny C x C inverse is computed on the host
and replicated to every core; activations are sharded along batch N (32 -> 4
per core). Each core runs a dense [K=256] x [M=256] x [N_free=4*4096] matmul.

The kernel is HBM-bandwidth-bound (~358 GB/s per NeuronCore), so all wire
traffic is fp16: activations are cast to fp16 on the host, the matmul runs
fp16 x fp16 -> fp32 PSUM, and the output is written back as fp16 and upcast
to fp32 on the host. This halves HBM traffic vs fp32 (16.8 MB/core total)
for a ~47 us DMA floor; fp16's 11-bit mantissa keeps rel err ~1e-3, far
under the 2e-2 gate (the old fp32r path also had an 11-bit mantissa).

  - weights held in SBUF as four 128x128 views of one [128, 512] tile
  - activations DMA'd in as [128, 2048] fp16 chunks (0.5 MB) on the SP
    HWDGE ring
  - PE matmul fp16, free dim 512, PSUM fp32
  - PSUM -> SBUF evacuation (with fp32->fp16 cast) alternating
    ScalarE / VectorE
  - output DMA'd out as [128, 2048] fp16 chunks on the ACT HWDGE ring, so
    read and write streams overlap
"""

import numpy as np

import concourse.bass as bass
import concourse.bacc as bacc
import concourse.mybir as mybir
import concourse.tile as tile
from concourse.bass_utils import run_bass_kernel_spmd

N, C, H, W = 32, 256, 64, 64
HW = H * W  # 4096
NCORES = 8
NB = N // NCORES  # batches per core
P = 128  # partitions
FD = 512  # matmul free dim (one fp32 PSUM bank)

MM_DT = mybir.dt.float16


def _build_w(inhibition_filter: np.ndarray) -> np.ndarray:
    """Replicates reference._pad_roll + _circulant + inv(I - tpl) in numpy."""
    filt = np.asarray(inhibition_filter, dtype=np.float32)
    scope = filt.shape[0]
    pad_left = (C - scope) // 2
    padded = np.zeros(C, np.float32)
    padded[pad_left : pad_left + scope] = filt
    kernel = np.roll(padded, C // 2 + 1)
    idx = (np.arange(C)[None, :] - np.arange(C)[:, None]) % C
    tpl = kernel[idx].astype(np.float64)
    w = np.linalg.inv(np.eye(C, dtype=np.float64) - tpl)
    return np.ascontiguousarray(w.astype(np.float32))


def _body(tc: tile.TileContext, out, act, w):
    # In-DMAs ride the SP HWDGE ring (nc.sync), out-DMAs the ACT ring
    # (nc.scalar) so input and output streams don't serialize on one FIFO
    # ring.
    nc = tc.nc
    CH = 2048  # chunk width, 4 matmul slices per chunk
    NCH = HW // CH  # 2 chunks
    JPC = CH // FD  # 4 matmul free-dim slices per chunk
    with (
        tc.tile_pool(name="wpool", bufs=1) as wpool,
        tc.tile_pool(name="apool", bufs=3) as apool,
        tc.tile_pool(name="opool", bufs=3) as opool,
        tc.tile_pool(name="psum", bufs=8, space="PSUM") as pspool,
    ):
        # One DMA for all four 128x128 weight tiles, on the ACT ring (idle at
        # start) so the SP ring's activation stream isn't stuck behind the
        # fixed completion latency of four tiny transfers.
        wtile = wpool.tile([P, 2 * C], MM_DT, tag="w", name="wtile")
        for k in range(2):
            nc.scalar.dma_start(
                out=wtile[:, k * C : (k + 1) * C], in_=w[k * P : (k + 1) * P, :]
            )
        wt = [
            [wtile[:, k * C + m * P : k * C + (m + 1) * P] for m in range(2)]
            for k in range(2)
        ]

        for n in range(NB):
            a = {}
            for c in range(NCH):
                for k in range(2):
                    a[k, c] = apool.tile([P, CH], MM_DT, tag=f"a{k}{c}", name=f"a{k}{c}")
                    nc.sync.dma_start(
                        out=a[k, c][:],
                        in_=act[n, k * P : (k + 1) * P, c * CH : (c + 1) * CH],
                    )
            for c in range(NCH):
                for m in range(2):
                    o = opool.tile([P, CH], MM_DT, tag=f"o{m}{c}", name=f"o{m}{c}", bufs=3 if c == 0 else 2)
                    for jj in range(JPC):
                        ps = pspool.tile([P, FD], mybir.dt.float32)
                        nc.tensor.matmul(
                            ps[:],
                            lhsT=wt[0][m],
                            rhs=a[0, c][:, jj * FD : (jj + 1) * FD],
                            start=True,
                            stop=False,
                        )
                        nc.tensor.matmul(
                            ps[:],
                            lhsT=wt[1][m],
                            rhs=a[1, c][:, jj * FD : (jj + 1) * FD],
                            start=False,
                            stop=True,
                        )
                        if jj % 2 == 0:
                            nc.scalar.copy(o[:, jj * FD : (jj + 1) * FD], ps[:])
                        else:
                            nc.vector.tensor_copy(o[:, jj * FD : (jj + 1) * FD], ps[:])
                    nc.scalar.dma_start(
                        out=out[n, m * P : (m + 1) * P, c * CH : (c + 1) * CH],
                        in_=o[:],
                    )


_NC_CACHE = None


def _get_nc():
    global _NC_CACHE
    if _NC_CACHE is None:
        nc = bacc.Bacc(
            "TRN2", debug=False, enable_asserts=False, enable_partition_id=False
        )
        act = nc.dram_tensor("act", [NB, C, HW], MM_DT, kind="ExternalInput").ap()
        w = nc.dram_tensor("w", [C, C], MM_DT, kind="ExternalInput").ap()
        out = nc.dram_tensor("out", [NB, C, HW], MM_DT, kind="ExternalOutput").ap()
        with tile.TileContext(nc) as tc:
            _body(tc, out, act, w)
        nc.compile()
        _NC_CACHE = nc
    return _NC_CACHE


def _run(activations: np.ndarray, w: np.ndarray, trace: bool = False):
    acts = (
        np.ascontiguousarray(activations, dtype=np.float32)
        .astype(np.float16)
        .reshape(NCORES, NB, C, HW)
    )
    w16 = w.astype(np.float16)
    in_maps = [{"act": acts[i], "w": w16} for i in range(NCORES)]
    nc = _get_nc()
    res = run_bass_kernel_spmd(nc, in_maps, list(range(NCORES)), trace=trace)
    out = np.concatenate([res.results[i]["out"] for i in range(NCORES)], axis=0)
    return out.astype(np.float32).reshape(N, C, H, W), res


def kernel(activations: np.ndarray, inhibition_filter: np.ndarray) -> np.ndarray:
    w = _build_w(inhibition_filter)
    out, _ = _run(activations, w, trace=False)
    return out
